# revision 19
# baseline (speedup 1.0000x reference)
r"""Circulant layer kernel for Trainium2 (8 NeuronCores) — v7.

Math (same as v2): reference computes mv1 + mv2 = 2 * circconv(d, b)
with d = des @ K, b = body @ K.  Real-input half-spectrum DFT: cores
0..7 own freqs f = 64c..64c+63; Nyquist f=512 rides core 0's slot-0
imaginary column with the generalized 3-product inverse (G3).

v7 vs v3: the input stream is ordered by when each tensor is needed.
dbt (needed at S2 partial 1, ~24us) rides the kt h1 HWDGE DMAs; g3a/g3b
(needed only at S4, ~26us) move to the slow SWDGE queue behind cc.
Keeping non-DMA engines QUIET during the stream matters: junk matmuls /
generation ALU measurably throttle DMA ingress (245 -> 140GB/s).

v3 structural changes vs v2 (40.9us -> 38.1 measured):
  * K^T streams on BOTH hardware DMA queues (SP + ACT), split by
    j-chunk pairs and k-halves; cc/dbt ride the gpsimd SWDGE queue.
    (v2 put all of kt on one queue at ~190GB/s — the single-queue
    stream, not PE, set the critical path.)
  * k-half phasing: S1 (KC^T = CC^T K^T) accumulates k-half 0 in PSUM
    bank 0 and k-half 1 in bank 1, so T1/S2 for half 0 run while
    half 1 is still streaming in.
  * The pointwise spectral products are computed directly in [s, b]
    layout from S2's output (DVE ops with partition-base-shifted
    operands — verified on HW), eliminating T2, T3 and their staging
    copies entirely:
      ptA[p, b]        = db[p, b] * db[p, B+b]          (p = 0..127)
      ptC2[p, b]       = db[p, b] * db[(p+64)%128, B+b] (two half ops)
    ptA/ptC2 feed S4 as stationaries with G3a / duplicated-C moving.
  * S4 + cast + store issue per 512-col PSUM bank as soon as ready.

Fixed costs measured by probe: ~8.3us preamble, ~2us DMA issue->land,
~2us store issue->tick, ~8.3us after last store tick.
"""

import numpy as np
import ml_dtypes

import concourse.bass as bass
import concourse.mybir as mybir
import concourse.tile as tile
from concourse.bass_utils import run_bass_kernel_spmd
from concourse.tile_rust import add_dep_helper

B = 128        # batch
D_IN = 1024    # input feature dim (contraction k)
N = 1024       # output feature dim (conv length j)
N_CORES = 8
FPC = 64       # complex frequency slots per core
S = 2 * FPC    # 128 freq columns per core: [0:64]=re(cos), [64:128]=im(-sin)

F32 = mybir.dt.float32
BF16 = mybir.dt.bfloat16

LAST_RESULT = None
_nc_cache = {}


def _build_nc():
    nc = bass.Bass(target_bir_lowering=True)

    # --- DRAM params (bf16 packed two-per-f32-word) ---
    # SP queue: [id | kt h0 c0c1] [kt h0 c2c3] [kt h1 c0c1] [kt h1 c2c3] [g3a]
    # ACT queue: [kt h0 c4c5] [kt h0 c6c7] [kt h1 c4c5] [kt h1 c6c7] [g3b2]
    # GP queue: [cc] [dbt]
    # each kt (c,h) block: [128, 512] bf16 = 256 f32 words; pairs = 512 words
    sp1 = nc.declare_dram_parameter("sp1", [128, 64 + 1024], F32, False)
    sp2 = nc.declare_dram_parameter("sp2", [128, 1024], F32, False)
    ac1 = nc.declare_dram_parameter("ac1", [128, 1024], F32, False)
    ac2 = nc.declare_dram_parameter("ac2", [128, 2048], F32, False)
    cc = nc.declare_dram_parameter("cc", [128, 512], F32, False)
    g3a = nc.declare_dram_parameter("g3a", [128, 512], F32, False)
    g3b = nc.declare_dram_parameter("g3b", [64, 512], F32, False)
    out = nc.declare_dram_parameter("out", [B, N // 2], F32, isOutput=True)

    with tile.TileContext(nc) as tc:
        with (
            tc.tile_pool(name="main", bufs=1) as pool,
            tc.tile_pool(name="psum", bufs=1, space="PSUM") as pp,
        ):
            # ---- input DMAs, phase-ordered per queue ----
            sp1_sb = pool.tile([128, 64 + 1024], F32, tag="sp1", name="sp1")
            sp2_sb = pool.tile([128, 1024], F32, tag="sp2", name="sp2")
            ac1_sb = pool.tile([128, 1024], F32, tag="ac1", name="ac1")
            ac2_sb = pool.tile([128, 2048], F32, tag="ac2", name="ac2")
            cc_sb = pool.tile([128, 512], F32, tag="cc", name="cc")
            g3a_sb = pool.tile([128, 512], F32, tag="g3a", name="g3a")
            g3b_sb = pool.tile([128, 512], F32, tag="g3b", name="g3b")

            in_dmas = []
            in_dmas.append(nc.sync.dma_start(sp1_sb[:], sp1[:, :]))
            in_dmas.append(nc.sync.dma_start(sp2_sb[:], sp2[:, :]))
            in_dmas.append(nc.sync.dma_start(g3a_sb[:], g3a[:, :]))
            in_dmas.append(nc.scalar.dma_start(ac1_sb[:], ac1[:, :]))
            in_dmas.append(nc.scalar.dma_start(ac2_sb[:], ac2[:, :]))
            in_dmas.append(nc.gpsimd.dma_start(cc_sb[:], cc[:, :]))
            in_dmas.append(nc.gpsimd.dma_start(g3b_sb[0:64, :], g3b[:, :]))

            # bf16 views
            id_v = sp1_sb.bitcast(BF16)[:, 0:128]
            # kt[c][h] -> [128, 512] bf16 view
            sp1v = sp1_sb.bitcast(BF16)
            sp2v = sp2_sb.bitcast(BF16)
            ac1v = ac1_sb.bitcast(BF16)
            ac2v = ac2_sb.bitcast(BF16)
            ktv = {}
            for c in range(4):
                ktv[(c, 0)] = sp1v[:, 128 + c * 512:128 + (c + 1) * 512]
                ktv[(c, 1)] = sp2v[:, c * 512:(c + 1) * 512]
                ktv[(4 + c, 0)] = ac1v[:, c * 512:(c + 1) * 512]
                ktv[(4 + c, 1)] = ac2v[:, c * 512:(c + 1) * 512]
            g3a_v = g3a_sb.bitcast(BF16)          # [128, 1024]
            # g3b = [C; C]: only rows 0-63 are DMA'd; duplicate on DVE
            g3b_v = g3b_sb.bitcast(BF16)          # [128, 1024]
            cc_v = cc_sb.bitcast(BF16).rearrange(
                "p (c s) -> p c s", c=8)          # [128, 8, 128]
            # dbt rides the tail of ac2 (all 8 k-chunks)
            dbt_lo = ac2v[:, 2048:3072].rearrange(
                "p (c w) -> p c w", c=4)          # [128, 4, 256]
            dbt_hi = ac2v[:, 3072:4096].rearrange(
                "p (c w) -> p c w", c=4)

            # ---- PSUM layout ----
            ps_kc0 = pp.tile([128, 512], F32, tag="pskc0", name="pskc0")
            ps_kc1 = pp.tile([128, 512], F32, tag="pskc1", name="pskc1")
            ps_db = pp.tile([128, 2 * B], F32, tag="psdb", name="psdb")
            trall = pp.tile([128, 4, 128], BF16, tag="trall", name="trall")
            trall2 = pp.tile([128, 4, 128], BF16, tag="trall2", name="trall2")
            ps_out_lo = pp.tile([128, 512], F32, tag="psoutl", name="psoutl")
            ps_out_hi = pp.tile([128, 512], F32, tag="psouth", name="psouth")
            ps_junk = pp.tile([128, 512], F32, tag="psjunk", name="psjunk")

            # ---- PE warmup: junk matmuls into ps_out (S4 overwrites) ----
            wz = pool.tile([128, 640], BF16, tag="wz", name="wz")
            memset_h = nc.gpsimd.memset(wz[:], 0.0)
            for w in range(4):
                nc.tensor.matmul(ps_junk[:], wz[:, :128], wz[:, 128:640],
                                 start=True, stop=True)

            # ---- S1 phase 0: ps_kc0[s, k0:512] = sum_j cc[j,s]^T kt[j, h0] ----
            # mm order follows expected landing: SP pair (0,1), ACT (4,5),
            # SP (2,3), ACT (6,7)
            h0_order = [0, 1, 4, 5, 2, 3, 6, 7]
            for i, c in enumerate(h0_order):
                nc.tensor.matmul(ps_kc0[:], cc_v[:, c, :], ktv[(c, 0)],
                                 start=(i == 0), stop=(i == 7))

            # ---- T1 phase 0: transpose KC^T[:, 0:512] -> kc chunks 0..3 ----
            kcT0 = pool.tile([128, 512], BF16, tag="kcT0", name="kcT0")
            nc.scalar.copy(kcT0[:], ps_kc0[:])
            tr_h0_last = None
            for c in range(4):
                tr_h0_last = nc.tensor.transpose(
                    trall[:, c, :], kcT0[:, c * 128:(c + 1) * 128], id_v)
            kc_lo = pool.tile([128, 4, 128], BF16, tag="kclo", name="kclo")
            nc.vector.tensor_copy(
                kc_lo[:].rearrange("p c s -> p (c s)"),
                trall[:].rearrange("p c s -> p (c s)"))

            # ---- S1 phase 1 (first half): overlap with S2p0 setup ----
            # pin the h0 transposes before S1h1 so they fill the PE idle
            # window while kt h1 is still streaming (tile otherwise
            # reorders them after S1h1)
            h1_order = [0, 1, 4, 5, 2, 3, 6, 7]
            h1_first = None
            for i, c in enumerate(h1_order[:2]):
                h = nc.tensor.matmul(ps_kc1[:], cc_v[:, c, :], ktv[(c, 1)],
                                     start=(i == 0), stop=False)
                if i == 0:
                    h1_first = h
                    add_dep_helper(h1_first.ins, tr_h0_last.ins, sync=False,
                                   reason="order: T1h0 before S1h1")

            # ---- S2 partial 0: ps_db += kc[k0 chunks] @ dbt ----
            for c in range(4):
                nc.tensor.matmul(ps_db[:], kc_lo[:, c, :], dbt_lo[:, c, :],
                                 start=(c == 0), stop=False)

            # ---- S1 phase 1 (rest) ----
            for i, c in enumerate(h1_order[2:]):
                nc.tensor.matmul(ps_kc1[:], cc_v[:, c, :], ktv[(c, 1)],
                                 start=False, stop=(i == 5))

            # ---- T1 phase 1 ----
            kcT1 = pool.tile([128, 512], BF16, tag="kcT1", name="kcT1")
            nc.scalar.copy(kcT1[:], ps_kc1[:])
            for c in range(4):
                nc.tensor.transpose(trall2[:, c, :],
                                    kcT1[:, c * 128:(c + 1) * 128], id_v)
            # separate PSUM tile (trall2): a second read of the same PSUM
            # tile needs two sync waits, which compute instrs can't encode
            kc_hi = pool.tile([128, 4, 128], BF16, tag="kchi", name="kchi")
            nc.vector.tensor_copy(
                kc_hi[:].rearrange("p c s -> p (c s)"),
                trall2[:].rearrange("p c s -> p (c s)"))

            # ---- S2 partial 1 ----
            s2p1_last = None
            for c in range(4):
                s2p1_last = nc.tensor.matmul(ps_db[:], kc_hi[:, c, :],
                                             dbt_hi[:, c, :],
                                             start=False, stop=(c == 3))

            # duplicate C into partitions 64-127 (partition-shifted copy)
            nc.vector.tensor_copy(g3b_v[64:128, :], g3b_v[0:64, :])

            # ---- PW in [s, b] layout (no transposes) ----
            db_sb = pool.tile([128, 2 * B], BF16, tag="db", name="db")
            nc.vector.tensor_copy(db_sb[:], ps_db[:])
            # partition-swapped copy of the B half (tensor_tensor requires
            # same start partition on all APs; tensor_copy does not)
            dbsw = pool.tile([128, B], BF16, tag="dbsw", name="dbsw")
            nc.vector.tensor_copy(dbsw[0:64, :], db_sb[64:128, B:2 * B])
            nc.vector.tensor_copy(dbsw[64:128, :], db_sb[0:64, B:2 * B])
            ptA = pool.tile([128, B], BF16, tag="ptA", name="ptA")
            ptC2 = pool.tile([128, B], BF16, tag="ptC2", name="ptC2")
            nc.vector.tensor_mul(ptA[:], db_sb[:, 0:B], db_sb[:, B:2 * B])
            nc.vector.tensor_mul(ptC2[:], db_sb[:, 0:B], dbsw[:])

            # hold the PE clock through the PW (DVE) window; pinned after
            # S2p1 so these cannot float back into the DMA stream
            jh = None
            for w in range(3):
                jh = nc.tensor.matmul(ps_junk[:, 0:256], wz[:, :128],
                                      wz[:, 128:384], start=True, stop=True)
                if w == 0:
                    add_dep_helper(jh.ins, s2p1_last.ins, sync=False,
                                   reason="order: junk after S2p1")

            # ---- S4 per bank: out[b, j] = ptA^T G3a + ptC2^T G3b2 ----
            out_lo = pool.tile([128, 512], BF16, tag="outlo", name="outlo")
            out_hi = pool.tile([128, 512], BF16, tag="outhi", name="outhi")
            stores = []
            nc.tensor.matmul(ps_out_lo[:], ptA[:], g3a_v[:, 0:512],
                             start=True, stop=False)
            nc.tensor.matmul(ps_out_lo[:], ptC2[:], g3b_v[:, 0:512],
                             start=False, stop=True)
            cp_lo = nc.scalar.copy(out_lo[:], ps_out_lo[:])
            stores.append(nc.sync.dma_start(out[:, :256],
                                            out_lo.bitcast(F32)[:, :]))
            nc.tensor.matmul(ps_out_hi[:], ptA[:], g3a_v[:, 512:1024],
                             start=True, stop=False)
            last_mm = nc.tensor.matmul(ps_out_hi[:], ptC2[:],
                                       g3b_v[:, 512:1024],
                                       start=False, stop=True)
            cp_hi = nc.vector.tensor_copy(out_hi[:], ps_out_hi[:])
            stores.append(nc.scalar.dma_start(out[:, 256:],
                                              out_hi.bitcast(F32)[:, :]))

            # ---- tail: absorb every outstanding tick into SP's clock ----
            prev = None
            for dep in [*in_dmas, memset_h, *stores, last_mm, cp_lo, cp_hi]:
                dr = nc.sync.drain(fusable=False)
                add_dep_helper(dr.ins, dep.ins, sync=True,
                               reason="tail: absorb tick into SP clock")
                if prev is not None:
                    add_dep_helper(dr.ins, prev.ins, sync=False,
                                   reason="tail: keep drain chain ordered")
                prev = dr

    return nc


def _bf16_pack(a):
    """float32 (P, W) -> bf16 packed two-per-word as float32 (P, W//2)."""
    bf = np.ascontiguousarray(np.asarray(a, np.float32).astype(ml_dtypes.bfloat16))
    return bf.view(np.uint8).reshape(bf.shape[0], -1).view(np.float32)


def _partition_pack(a):
    """(n*128, W) -> (128, n*W): row p = concat of chunk rows p."""
    r, w = a.shape
    n = r // 128
    return np.ascontiguousarray(
        a.reshape(n, 128, w).transpose(1, 0, 2).reshape(128, n * w))


def _constants():
    """Per-core CC [N, S], G3a [128, N], G3b2 [128, N] float32."""
    j = np.arange(N, dtype=np.float64)
    alt = np.cos(np.pi * j)                     # (-1)^j
    ccs, g3as, g3bs = [], [], []
    for c in range(N_CORES):
        f = np.arange(c * FPC, (c + 1) * FPC, dtype=np.float64)
        ang = 2.0 * np.pi * np.outer(j, f) / N             # (j, t)
        cc_re = np.cos(ang)
        cc_im = -np.sin(ang)
        angT = ang.T                                        # (t, j)
        w = 4.0 / N
        A = w * np.cos(angT)                                # m0 rows
        Bm = -w * np.cos(angT)                              # m1 rows
        C = -w * np.sin(angT)                               # mC rows
        if c == 0:
            cc_im[:, 0] = alt                               # f=512 cos column
            A[0, :] = 2.0 / N                               # m0 = D0*B0
            Bm[0, :] = (2.0 / N) * alt                      # m1 = D512*B512
            C[0, :] = 0.0
        cc_full = np.concatenate([cc_re, cc_im], axis=1)    # (N, 128)
        ccs.append(np.ascontiguousarray(cc_full, np.float32))
        g3as.append(np.ascontiguousarray(
            np.concatenate([A, Bm], axis=0), np.float32))       # (128, N)
        g3bs.append(np.ascontiguousarray(
            np.concatenate([C, C], axis=0), np.float32))        # (128, N)
    return ccs, g3as, g3bs


def kernel(des, body, kernel):
    global LAST_RESULT
    K = np.asarray(kernel, dtype=np.float32)
    des = np.asarray(des, dtype=np.float32)
    body = np.asarray(body, dtype=np.float32)

    # K^T as bf16 blocks: block (c, h) = K^T[c*128:(c+1)*128, h*512:(h+1)*512]
    ktb = K.T.astype(ml_dtypes.bfloat16)                # (1024 j, 1024 k)
    def ktpk(c, h):
        blk = np.ascontiguousarray(
            ktb[c * 128:(c + 1) * 128, h * 512:(h + 1) * 512], np.float32)
        return _bf16_pack(blk)                          # (128, 256) words

    id_pk = _bf16_pack(np.eye(128, dtype=np.float32))   # (128, 64) words
    dbt_np = np.concatenate([des.T, body.T], axis=1)    # (1024, 256)
    dbt_pk = _partition_pack(_bf16_pack(dbt_np))        # (128, 1024) words

    ccs, g3as, g3bs = _constants()
    in_maps = []
    for c in range(N_CORES):
        cc_pk = _partition_pack(_bf16_pack(ccs[c]))     # (128, 512) words
        m = {
            "sp1": np.ascontiguousarray(np.concatenate(
                [id_pk, ktpk(0, 0), ktpk(1, 0), ktpk(2, 0), ktpk(3, 0)],
                axis=1)),
            "sp2": np.ascontiguousarray(np.concatenate(
                [ktpk(0, 1), ktpk(1, 1), ktpk(2, 1), ktpk(3, 1)], axis=1)),
            "ac1": np.ascontiguousarray(np.concatenate(
                [ktpk(4, 0), ktpk(5, 0), ktpk(6, 0), ktpk(7, 0)], axis=1)),
            "ac2": np.ascontiguousarray(np.concatenate(
                [ktpk(4, 1), ktpk(5, 1), ktpk(6, 1), ktpk(7, 1),
                 dbt_pk], axis=1)),
            "cc": cc_pk,
            "g3a": np.ascontiguousarray(_bf16_pack(g3as[c])),
            "g3b": np.ascontiguousarray(_bf16_pack(g3bs[c][0:64])),
        }
        in_maps.append(m)

    if "nc" not in _nc_cache:
        _nc_cache["nc"] = _build_nc()
    nc = _nc_cache["nc"]

    res = run_bass_kernel_spmd(nc, in_maps, list(range(N_CORES)))
    LAST_RESULT = res
    out = np.zeros((B, N), dtype=np.float32)
    for r in res.results:
        w = np.ascontiguousarray(np.asarray(r["out"], np.float32))
        bf = w.view(np.uint8).reshape(B, -1).view(ml_dtypes.bfloat16)
        out += bf.astype(np.float32)
    return out


# revision 20
# speedup vs baseline: 1.0642x; 1.0642x over previous
r"""Circulant layer kernel for Trainium2 (8 NeuronCores) — v7.

Math (same as v2): reference computes mv1 + mv2 = 2 * circconv(d, b)
with d = des @ K, b = body @ K.  Real-input half-spectrum DFT: cores
0..7 own freqs f = 64c..64c+63; Nyquist f=512 rides core 0's slot-0
imaginary column with the generalized 3-product inverse (G3).

v7 vs v3: the input stream is ordered by when each tensor is needed.
dbt (needed at S2 partial 1, ~24us) rides the kt h1 HWDGE DMAs; g3a/g3b
(needed only at S4, ~26us) move to the slow SWDGE queue behind cc.
Keeping non-DMA engines QUIET during the stream matters: junk matmuls /
generation ALU measurably throttle DMA ingress (245 -> 140GB/s).

v3 structural changes vs v2 (40.9us -> 38.1 measured):
  * K^T streams on BOTH hardware DMA queues (SP + ACT), split by
    j-chunk pairs and k-halves; cc/dbt ride the gpsimd SWDGE queue.
    (v2 put all of kt on one queue at ~190GB/s — the single-queue
    stream, not PE, set the critical path.)
  * k-half phasing: S1 (KC^T = CC^T K^T) accumulates k-half 0 in PSUM
    bank 0 and k-half 1 in bank 1, so T1/S2 for half 0 run while
    half 1 is still streaming in.
  * The pointwise spectral products are computed directly in [s, b]
    layout from S2's output (DVE ops with partition-base-shifted
    operands — verified on HW), eliminating T2, T3 and their staging
    copies entirely:
      ptA[p, b]        = db[p, b] * db[p, B+b]          (p = 0..127)
      ptC2[p, b]       = db[p, b] * db[(p+64)%128, B+b] (two half ops)
    ptA/ptC2 feed S4 as stationaries with G3a / duplicated-C moving.
  * S4 + cast + store issue per 512-col PSUM bank as soon as ready.

Fixed costs measured by probe: ~8.3us preamble, ~2us DMA issue->land,
~2us store issue->tick, ~8.3us after last store tick.
"""

import numpy as np
import ml_dtypes

import concourse.bass as bass
import concourse.mybir as mybir
import concourse.tile as tile
from concourse.bass_utils import run_bass_kernel_spmd
from concourse.tile_rust import add_dep_helper

B = 128        # batch
D_IN = 1024    # input feature dim (contraction k)
N = 1024       # output feature dim (conv length j)
N_CORES = 8
FPC = 64       # complex frequency slots per core
S = 2 * FPC    # 128 freq columns per core: [0:64]=re(cos), [64:128]=im(-sin)

F32 = mybir.dt.float32
BF16 = mybir.dt.bfloat16

LAST_RESULT = None
_nc_cache = {}


def _build_nc():
    nc = bass.Bass(target_bir_lowering=True)

    # --- DRAM params (bf16 packed two-per-f32-word) ---
    # SP queue: [id | kt h0 c0c1] [kt h0 c2c3] [kt h1 c0c1] [kt h1 c2c3] [g3a]
    # ACT queue: [kt h0 c4c5] [kt h0 c6c7] [kt h1 c4c5] [kt h1 c6c7] [g3b2]
    # GP queue: [cc] [dbt]
    # each kt (c,h) block: [128, 512] bf16 = 256 f32 words; pairs = 512 words
    sp1 = nc.declare_dram_parameter("sp1", [128, 64 + 1024], F32, False)
    sp2 = nc.declare_dram_parameter("sp2", [128, 1024], F32, False)
    ac1 = nc.declare_dram_parameter("ac1", [128, 1024], F32, False)
    ac2 = nc.declare_dram_parameter("ac2", [128, 2048], F32, False)
    cc = nc.declare_dram_parameter("cc", [128, 512], F32, False)
    g3a = nc.declare_dram_parameter("g3a", [128, 512], F32, False)
    g3b = nc.declare_dram_parameter("g3b", [64, 512], F32, False)
    out = nc.declare_dram_parameter("out", [B, N // 2], F32, isOutput=True)

    with tile.TileContext(nc) as tc:
        with (
            tc.tile_pool(name="main", bufs=1) as pool,
            tc.tile_pool(name="psum", bufs=1, space="PSUM") as pp,
        ):
            # ---- input DMAs, phase-ordered per queue ----
            sp1_sb = pool.tile([128, 64 + 1024], F32, tag="sp1", name="sp1")
            sp2_sb = pool.tile([128, 1024], F32, tag="sp2", name="sp2")
            ac1_sb = pool.tile([128, 1024], F32, tag="ac1", name="ac1")
            ac2_sb = pool.tile([128, 2048], F32, tag="ac2", name="ac2")
            cc_sb = pool.tile([128, 512], F32, tag="cc", name="cc")
            g3a_sb = pool.tile([128, 512], F32, tag="g3a", name="g3a")
            g3b_sb = pool.tile([128, 512], F32, tag="g3b", name="g3b")

            in_dmas = []
            in_dmas.append(nc.sync.dma_start(sp1_sb[:], sp1[:, :]))
            in_dmas.append(nc.sync.dma_start(sp2_sb[:], sp2[:, :]))
            in_dmas.append(nc.sync.dma_start(g3a_sb[:], g3a[:, :]))
            in_dmas.append(nc.scalar.dma_start(ac1_sb[:], ac1[:, :]))
            in_dmas.append(nc.scalar.dma_start(ac2_sb[:], ac2[:, :]))
            in_dmas.append(nc.gpsimd.dma_start(cc_sb[:], cc[:, :]))
            in_dmas.append(nc.gpsimd.dma_start(g3b_sb[0:64, :], g3b[:, :]))

            # bf16 views
            id_v = sp1_sb.bitcast(BF16)[:, 0:128]
            # kt[c][h] -> [128, 512] bf16 view
            sp1v = sp1_sb.bitcast(BF16)
            sp2v = sp2_sb.bitcast(BF16)
            ac1v = ac1_sb.bitcast(BF16)
            ac2v = ac2_sb.bitcast(BF16)
            ktv = {}
            for c in range(4):
                ktv[(c, 0)] = sp1v[:, 128 + c * 512:128 + (c + 1) * 512]
                ktv[(c, 1)] = sp2v[:, c * 512:(c + 1) * 512]
                ktv[(4 + c, 0)] = ac1v[:, c * 512:(c + 1) * 512]
                ktv[(4 + c, 1)] = ac2v[:, c * 512:(c + 1) * 512]
            g3a_v = g3a_sb.bitcast(BF16)          # [128, 1024]
            # g3b = [C; C]: only rows 0-63 are DMA'd; duplicate on DVE
            g3b_v = g3b_sb.bitcast(BF16)          # [128, 1024]
            cc_v = cc_sb.bitcast(BF16).rearrange(
                "p (c s) -> p c s", c=8)          # [128, 8, 128]
            # dbt rides the tail of ac2 (all 8 k-chunks)
            dbt_lo = ac2v[:, 2048:3072].rearrange(
                "p (c w) -> p c w", c=4)          # [128, 4, 256]
            dbt_hi = ac2v[:, 3072:4096].rearrange(
                "p (c w) -> p c w", c=4)

            # ---- PSUM layout ----
            ps_kc0 = pp.tile([128, 512], F32, tag="pskc0", name="pskc0")
            ps_kc1 = pp.tile([128, 512], F32, tag="pskc1", name="pskc1")
            ps_db = pp.tile([128, 2 * B], F32, tag="psdb", name="psdb")
            trall = pp.tile([128, 4, 128], BF16, tag="trall", name="trall")
            trall2 = pp.tile([128, 4, 128], BF16, tag="trall2", name="trall2")
            ps_out_lo = pp.tile([128, 512], F32, tag="psoutl", name="psoutl")
            ps_out_hi = pp.tile([128, 512], F32, tag="psouth", name="psouth")
            ps_junk = pp.tile([128, 512], F32, tag="psjunk", name="psjunk")

            # ---- PE warmup: junk matmuls into ps_out (S4 overwrites) ----
            wz = pool.tile([128, 640], BF16, tag="wz", name="wz")
            memset_h = nc.gpsimd.memset(wz[:], 0.0)
            for w in range(4):
                nc.tensor.matmul(ps_junk[:], wz[:, :128], wz[:, 128:640],
                                 start=True, stop=True)

            # ---- S1 phase 0: ps_kc0[s, k0:512] = sum_j cc[j,s]^T kt[j, h0] ----
            # mm order follows expected landing: SP pair (0,1), ACT (4,5),
            # SP (2,3), ACT (6,7)
            h0_order = [0, 1, 4, 5, 2, 3, 6, 7]
            for i, c in enumerate(h0_order):
                nc.tensor.matmul(ps_kc0[:], cc_v[:, c, :], ktv[(c, 0)],
                                 start=(i == 0), stop=(i == 7))

            # ---- T1 phase 0: transpose KC^T[:, 0:512] -> kc chunks 0..3 ----
            kcT0 = pool.tile([128, 512], BF16, tag="kcT0", name="kcT0")
            nc.scalar.copy(kcT0[:], ps_kc0[:])
            tr_h0_last = None
            for c in range(4):
                tr_h0_last = nc.tensor.transpose(
                    trall[:, c, :], kcT0[:, c * 128:(c + 1) * 128], id_v)
            kc_lo = pool.tile([128, 4, 128], BF16, tag="kclo", name="kclo")
            nc.vector.tensor_copy(
                kc_lo[:].rearrange("p c s -> p (c s)"),
                trall[:].rearrange("p c s -> p (c s)"))

            # ---- S1 phase 1 (first half): overlap with S2p0 setup ----
            h1_order = [0, 1, 4, 5, 2, 3, 6, 7]
            for i, c in enumerate(h1_order[:2]):
                nc.tensor.matmul(ps_kc1[:], cc_v[:, c, :], ktv[(c, 1)],
                                 start=(i == 0), stop=False)

            # ---- S2 partial 0: ps_db += kc[k0 chunks] @ dbt ----
            for c in range(4):
                nc.tensor.matmul(ps_db[:], kc_lo[:, c, :], dbt_lo[:, c, :],
                                 start=(c == 0), stop=False)

            # ---- S1 phase 1 (rest) ----
            for i, c in enumerate(h1_order[2:]):
                nc.tensor.matmul(ps_kc1[:], cc_v[:, c, :], ktv[(c, 1)],
                                 start=False, stop=(i == 5))

            # ---- T1 phase 1 ----
            kcT1 = pool.tile([128, 512], BF16, tag="kcT1", name="kcT1")
            nc.scalar.copy(kcT1[:], ps_kc1[:])
            for c in range(4):
                nc.tensor.transpose(trall2[:, c, :],
                                    kcT1[:, c * 128:(c + 1) * 128], id_v)
            # separate PSUM tile (trall2): a second read of the same PSUM
            # tile needs two sync waits, which compute instrs can't encode
            kc_hi = pool.tile([128, 4, 128], BF16, tag="kchi", name="kchi")
            nc.vector.tensor_copy(
                kc_hi[:].rearrange("p c s -> p (c s)"),
                trall2[:].rearrange("p c s -> p (c s)"))

            # ---- S2 partial 1 ----
            s2p1_last = None
            for c in range(4):
                s2p1_last = nc.tensor.matmul(ps_db[:], kc_hi[:, c, :],
                                             dbt_hi[:, c, :],
                                             start=False, stop=(c == 3))

            # duplicate C into partitions 64-127 (partition-shifted copy)
            nc.vector.tensor_copy(g3b_v[64:128, :], g3b_v[0:64, :])

            # ---- PW in [s, b] layout (no transposes) ----
            db_sb = pool.tile([128, 2 * B], BF16, tag="db", name="db")
            nc.vector.tensor_copy(db_sb[:], ps_db[:])
            # partition-swapped copy of the B half (tensor_tensor requires
            # same start partition on all APs; tensor_copy does not)
            dbsw = pool.tile([128, B], BF16, tag="dbsw", name="dbsw")
            nc.vector.tensor_copy(dbsw[0:64, :], db_sb[64:128, B:2 * B])
            nc.vector.tensor_copy(dbsw[64:128, :], db_sb[0:64, B:2 * B])
            ptA = pool.tile([128, B], BF16, tag="ptA", name="ptA")
            ptC2 = pool.tile([128, B], BF16, tag="ptC2", name="ptC2")
            nc.vector.tensor_mul(ptA[:], db_sb[:, 0:B], db_sb[:, B:2 * B])
            nc.vector.tensor_mul(ptC2[:], db_sb[:, 0:B], dbsw[:])

            # hold the PE clock through the PW (DVE) window; pinned after
            # S2p1 so these cannot float back into the DMA stream
            jh = None
            for w in range(3):
                jh = nc.tensor.matmul(ps_junk[:, 0:256], wz[:, :128],
                                      wz[:, 128:384], start=True, stop=True)
                if w == 0:
                    add_dep_helper(jh.ins, s2p1_last.ins, sync=False,
                                   reason="order: junk after S2p1")

            # ---- S4 per bank: out[b, j] = ptA^T G3a + ptC2^T G3b2 ----
            out_lo = pool.tile([128, 512], BF16, tag="outlo", name="outlo")
            out_hi = pool.tile([128, 512], BF16, tag="outhi", name="outhi")
            stores = []
            nc.tensor.matmul(ps_out_lo[:], ptA[:], g3a_v[:, 0:512],
                             start=True, stop=False)
            nc.tensor.matmul(ps_out_lo[:], ptC2[:], g3b_v[:, 0:512],
                             start=False, stop=True)
            cp_lo = nc.scalar.copy(out_lo[:], ps_out_lo[:])
            stores.append(nc.sync.dma_start(out[:, :256],
                                            out_lo.bitcast(F32)[:, :]))
            nc.tensor.matmul(ps_out_hi[:], ptA[:], g3a_v[:, 512:1024],
                             start=True, stop=False)
            last_mm = nc.tensor.matmul(ps_out_hi[:], ptC2[:],
                                       g3b_v[:, 512:1024],
                                       start=False, stop=True)
            cp_hi = nc.vector.tensor_copy(out_hi[:], ps_out_hi[:])
            stores.append(nc.scalar.dma_start(out[:, 256:],
                                              out_hi.bitcast(F32)[:, :]))

            # ---- tail: absorb every outstanding tick into SP's clock ----
            prev = None
            for dep in [*in_dmas, memset_h, *stores, last_mm, cp_lo, cp_hi]:
                dr = nc.sync.drain(fusable=False)
                add_dep_helper(dr.ins, dep.ins, sync=True,
                               reason="tail: absorb tick into SP clock")
                if prev is not None:
                    add_dep_helper(dr.ins, prev.ins, sync=False,
                                   reason="tail: keep drain chain ordered")
                prev = dr

    return nc


def _bf16_pack(a):
    """float32 (P, W) -> bf16 packed two-per-word as float32 (P, W//2)."""
    bf = np.ascontiguousarray(np.asarray(a, np.float32).astype(ml_dtypes.bfloat16))
    return bf.view(np.uint8).reshape(bf.shape[0], -1).view(np.float32)


def _partition_pack(a):
    """(n*128, W) -> (128, n*W): row p = concat of chunk rows p."""
    r, w = a.shape
    n = r // 128
    return np.ascontiguousarray(
        a.reshape(n, 128, w).transpose(1, 0, 2).reshape(128, n * w))


def _constants():
    """Per-core CC [N, S], G3a [128, N], G3b2 [128, N] float32."""
    j = np.arange(N, dtype=np.float64)
    alt = np.cos(np.pi * j)                     # (-1)^j
    ccs, g3as, g3bs = [], [], []
    for c in range(N_CORES):
        f = np.arange(c * FPC, (c + 1) * FPC, dtype=np.float64)
        ang = 2.0 * np.pi * np.outer(j, f) / N             # (j, t)
        cc_re = np.cos(ang)
        cc_im = -np.sin(ang)
        angT = ang.T                                        # (t, j)
        w = 4.0 / N
        A = w * np.cos(angT)                                # m0 rows
        Bm = -w * np.cos(angT)                              # m1 rows
        C = -w * np.sin(angT)                               # mC rows
        if c == 0:
            cc_im[:, 0] = alt                               # f=512 cos column
            A[0, :] = 2.0 / N                               # m0 = D0*B0
            Bm[0, :] = (2.0 / N) * alt                      # m1 = D512*B512
            C[0, :] = 0.0
        cc_full = np.concatenate([cc_re, cc_im], axis=1)    # (N, 128)
        ccs.append(np.ascontiguousarray(cc_full, np.float32))
        g3as.append(np.ascontiguousarray(
            np.concatenate([A, Bm], axis=0), np.float32))       # (128, N)
        g3bs.append(np.ascontiguousarray(
            np.concatenate([C, C], axis=0), np.float32))        # (128, N)
    return ccs, g3as, g3bs


def kernel(des, body, kernel):
    global LAST_RESULT
    K = np.asarray(kernel, dtype=np.float32)
    des = np.asarray(des, dtype=np.float32)
    body = np.asarray(body, dtype=np.float32)

    # K^T as bf16 blocks: block (c, h) = K^T[c*128:(c+1)*128, h*512:(h+1)*512]
    ktb = K.T.astype(ml_dtypes.bfloat16)                # (1024 j, 1024 k)
    def ktpk(c, h):
        blk = np.ascontiguousarray(
            ktb[c * 128:(c + 1) * 128, h * 512:(h + 1) * 512], np.float32)
        return _bf16_pack(blk)                          # (128, 256) words

    id_pk = _bf16_pack(np.eye(128, dtype=np.float32))   # (128, 64) words
    dbt_np = np.concatenate([des.T, body.T], axis=1)    # (1024, 256)
    dbt_pk = _partition_pack(_bf16_pack(dbt_np))        # (128, 1024) words

    ccs, g3as, g3bs = _constants()
    in_maps = []
    for c in range(N_CORES):
        cc_pk = _partition_pack(_bf16_pack(ccs[c]))     # (128, 512) words
        m = {
            "sp1": np.ascontiguousarray(np.concatenate(
                [id_pk, ktpk(0, 0), ktpk(1, 0), ktpk(2, 0), ktpk(3, 0)],
                axis=1)),
            "sp2": np.ascontiguousarray(np.concatenate(
                [ktpk(0, 1), ktpk(1, 1), ktpk(2, 1), ktpk(3, 1)], axis=1)),
            "ac1": np.ascontiguousarray(np.concatenate(
                [ktpk(4, 0), ktpk(5, 0), ktpk(6, 0), ktpk(7, 0)], axis=1)),
            "ac2": np.ascontiguousarray(np.concatenate(
                [ktpk(4, 1), ktpk(5, 1), ktpk(6, 1), ktpk(7, 1),
                 dbt_pk], axis=1)),
            "cc": cc_pk,
            "g3a": np.ascontiguousarray(_bf16_pack(g3as[c])),
            "g3b": np.ascontiguousarray(_bf16_pack(g3bs[c][0:64])),
        }
        in_maps.append(m)

    if "nc" not in _nc_cache:
        _nc_cache["nc"] = _build_nc()
    nc = _nc_cache["nc"]

    res = run_bass_kernel_spmd(nc, in_maps, list(range(N_CORES)))
    LAST_RESULT = res
    out = np.zeros((B, N), dtype=np.float32)
    for r in res.results:
        w = np.ascontiguousarray(np.asarray(r["out"], np.float32))
        bf = w.view(np.uint8).reshape(B, -1).view(ml_dtypes.bfloat16)
        out += bf.astype(np.float32)
    return out


# revision 21
# speedup vs baseline: 1.0710x; 1.0063x over previous
r"""Circulant layer kernel for Trainium2 (8 NeuronCores) — v7.

Math (same as v2): reference computes mv1 + mv2 = 2 * circconv(d, b)
with d = des @ K, b = body @ K.  Real-input half-spectrum DFT: cores
0..7 own freqs f = 64c..64c+63; Nyquist f=512 rides core 0's slot-0
imaginary column with the generalized 3-product inverse (G3).

v7 vs v3: the input stream is ordered by when each tensor is needed.
dbt (needed at S2 partial 1, ~24us) rides the kt h1 HWDGE DMAs; g3a/g3b
(needed only at S4, ~26us) move to the slow SWDGE queue behind cc.
Keeping non-DMA engines QUIET during the stream matters: junk matmuls /
generation ALU measurably throttle DMA ingress (245 -> 140GB/s).

v3 structural changes vs v2 (40.9us -> 38.1 measured):
  * K^T streams on BOTH hardware DMA queues (SP + ACT), split by
    j-chunk pairs and k-halves; cc/dbt ride the gpsimd SWDGE queue.
    (v2 put all of kt on one queue at ~190GB/s — the single-queue
    stream, not PE, set the critical path.)
  * k-half phasing: S1 (KC^T = CC^T K^T) accumulates k-half 0 in PSUM
    bank 0 and k-half 1 in bank 1, so T1/S2 for half 0 run while
    half 1 is still streaming in.
  * The pointwise spectral products are computed directly in [s, b]
    layout from S2's output (DVE ops with partition-base-shifted
    operands — verified on HW), eliminating T2, T3 and their staging
    copies entirely:
      ptA[p, b]        = db[p, b] * db[p, B+b]          (p = 0..127)
      ptC2[p, b]       = db[p, b] * db[(p+64)%128, B+b] (two half ops)
    ptA/ptC2 feed S4 as stationaries with G3a / duplicated-C moving.
  * S4 + cast + store issue per 512-col PSUM bank as soon as ready.

Fixed costs measured by probe: ~8.3us preamble, ~2us DMA issue->land,
~2us store issue->tick, ~8.3us after last store tick.
"""

import numpy as np
import ml_dtypes

import concourse.bass as bass
import concourse.mybir as mybir
import concourse.tile as tile
from concourse.bass_utils import run_bass_kernel_spmd
from concourse.tile_rust import add_dep_helper

B = 128        # batch
D_IN = 1024    # input feature dim (contraction k)
N = 1024       # output feature dim (conv length j)
N_CORES = 8
FPC = 64       # complex frequency slots per core
S = 2 * FPC    # 128 freq columns per core: [0:64]=re(cos), [64:128]=im(-sin)

F32 = mybir.dt.float32
BF16 = mybir.dt.bfloat16

LAST_RESULT = None
_nc_cache = {}


def _build_nc():
    nc = bass.Bass(target_bir_lowering=True)

    # --- DRAM params (bf16 packed two-per-f32-word) ---
    # SP queue: [id | kt h0 c0c1] [kt h0 c2c3] [kt h1 c0c1] [kt h1 c2c3] [g3a]
    # ACT queue: [kt h0 c4c5] [kt h0 c6c7] [kt h1 c4c5] [kt h1 c6c7] [g3b2]
    # GP queue: [cc] [dbt]
    # each kt (c,h) block: [128, 512] bf16 = 256 f32 words; pairs = 512 words
    sp1 = nc.declare_dram_parameter("sp1", [128, 64 + 1024], F32, False)
    sp2 = nc.declare_dram_parameter("sp2", [128, 1024], F32, False)
    ac1 = nc.declare_dram_parameter("ac1", [128, 1024], F32, False)
    ac2 = nc.declare_dram_parameter("ac2", [128, 2048], F32, False)
    cc = nc.declare_dram_parameter("cc", [128, 512], F32, False)
    g3a = nc.declare_dram_parameter("g3a", [128, 512], F32, False)
    g3b = nc.declare_dram_parameter("g3b", [64, 512], F32, False)
    out = nc.declare_dram_parameter("out", [B, N // 2], F32, isOutput=True)

    with tile.TileContext(nc) as tc:
        with (
            tc.tile_pool(name="main", bufs=1) as pool,
            tc.tile_pool(name="psum", bufs=1, space="PSUM") as pp,
        ):
            # ---- input DMAs, phase-ordered per queue ----
            sp1_sb = pool.tile([128, 64 + 1024], F32, tag="sp1", name="sp1")
            sp2_sb = pool.tile([128, 1024], F32, tag="sp2", name="sp2")
            ac1_sb = pool.tile([128, 1024], F32, tag="ac1", name="ac1")
            ac2_sb = pool.tile([128, 2048], F32, tag="ac2", name="ac2")
            cc_sb = pool.tile([128, 512], F32, tag="cc", name="cc")
            g3a_sb = pool.tile([128, 512], F32, tag="g3a", name="g3a")
            g3b_sb = pool.tile([128, 512], F32, tag="g3b", name="g3b")

            in_dmas = []
            in_dmas.append(nc.sync.dma_start(sp1_sb[:], sp1[:, :]))
            in_dmas.append(nc.sync.dma_start(sp2_sb[:], sp2[:, :]))
            in_dmas.append(nc.sync.dma_start(g3a_sb[:], g3a[:, :]))
            in_dmas.append(nc.scalar.dma_start(ac1_sb[:], ac1[:, :]))
            in_dmas.append(nc.scalar.dma_start(ac2_sb[:], ac2[:, :]))
            in_dmas.append(nc.gpsimd.dma_start(cc_sb[:], cc[:, :]))
            in_dmas.append(nc.gpsimd.dma_start(g3b_sb[0:64, :], g3b[:, :]))

            # bf16 views
            id_v = sp1_sb.bitcast(BF16)[:, 0:128]
            # kt[c][h] -> [128, 512] bf16 view
            sp1v = sp1_sb.bitcast(BF16)
            sp2v = sp2_sb.bitcast(BF16)
            ac1v = ac1_sb.bitcast(BF16)
            ac2v = ac2_sb.bitcast(BF16)
            ktv = {}
            for c in range(4):
                ktv[(c, 0)] = sp1v[:, 128 + c * 512:128 + (c + 1) * 512]
                ktv[(c, 1)] = sp2v[:, c * 512:(c + 1) * 512]
                ktv[(4 + c, 0)] = ac1v[:, c * 512:(c + 1) * 512]
                ktv[(4 + c, 1)] = ac2v[:, c * 512:(c + 1) * 512]
            g3a_v = g3a_sb.bitcast(BF16)          # [128, 1024]
            # g3b = [C; C]: only rows 0-63 are DMA'd; duplicate on DVE
            g3b_v = g3b_sb.bitcast(BF16)          # [128, 1024]
            cc_v = cc_sb.bitcast(BF16).rearrange(
                "p (c s) -> p c s", c=8)          # [128, 8, 128]
            # dbt rides the tail of ac2 (all 8 k-chunks)
            dbt_lo = ac2v[:, 2048:3072].rearrange(
                "p (c w) -> p c w", c=4)          # [128, 4, 256]
            dbt_hi = ac2v[:, 3072:4096].rearrange(
                "p (c w) -> p c w", c=4)

            # ---- PSUM layout ----
            ps_kc0 = pp.tile([128, 512], F32, tag="pskc0", name="pskc0")
            ps_kc1 = pp.tile([128, 512], F32, tag="pskc1", name="pskc1")
            ps_db = pp.tile([128, 2 * B], F32, tag="psdb", name="psdb")
            trall = pp.tile([128, 4, 128], BF16, tag="trall", name="trall")
            trall2 = pp.tile([128, 4, 128], BF16, tag="trall2", name="trall2")
            ps_out_lo = pp.tile([128, 512], F32, tag="psoutl", name="psoutl")
            ps_out_hi = pp.tile([128, 512], F32, tag="psouth", name="psouth")
            ps_junk = pp.tile([128, 512], F32, tag="psjunk", name="psjunk")

            # ---- PE warmup: junk matmuls into ps_out (S4 overwrites) ----
            wz = pool.tile([128, 640], BF16, tag="wz", name="wz")
            memset_h = nc.gpsimd.memset(wz[:], 0.0)
            for w in range(4):
                nc.tensor.matmul(ps_junk[:], wz[:, :128], wz[:, 128:640],
                                 start=True, stop=True)

            # ---- S1 phase 0: ps_kc0[s, k0:512] = sum_j cc[j,s]^T kt[j, h0] ----
            # mm order follows expected landing: SP pair (0,1), ACT (4,5),
            # SP (2,3), ACT (6,7)
            h0_order = [0, 1, 4, 5, 2, 3, 6, 7]
            for i, c in enumerate(h0_order):
                nc.tensor.matmul(ps_kc0[:], cc_v[:, c, :], ktv[(c, 0)],
                                 start=(i == 0), stop=(i == 7))

            # ---- T1 phase 0: transpose KC^T[:, 0:512] -> kc chunks 0..3 ----
            kcT0 = pool.tile([128, 512], BF16, tag="kcT0", name="kcT0")
            nc.scalar.copy(kcT0[:], ps_kc0[:])
            tr_h0_last = None
            for c in range(4):
                tr_h0_last = nc.tensor.transpose(
                    trall[:, c, :], kcT0[:, c * 128:(c + 1) * 128], id_v)
            kc_lo = pool.tile([128, 4, 128], BF16, tag="kclo", name="kclo")
            nc.vector.tensor_copy(
                kc_lo[:].rearrange("p c s -> p (c s)"),
                trall[:].rearrange("p c s -> p (c s)"))

            # ---- S1 phase 1 (first half): overlap with S2p0 setup ----
            h1_order = [0, 1, 4, 5, 2, 3, 6, 7]
            for i, c in enumerate(h1_order[:2]):
                nc.tensor.matmul(ps_kc1[:], cc_v[:, c, :], ktv[(c, 1)],
                                 start=(i == 0), stop=False)

            # ---- S2 partial 0: ps_db += kc[k0 chunks] @ dbt ----
            for c in range(4):
                nc.tensor.matmul(ps_db[:], kc_lo[:, c, :], dbt_lo[:, c, :],
                                 start=(c == 0), stop=False)

            # ---- S1 phase 1 (rest) ----
            for i, c in enumerate(h1_order[2:]):
                nc.tensor.matmul(ps_kc1[:], cc_v[:, c, :], ktv[(c, 1)],
                                 start=False, stop=(i == 5))

            # ---- T1 phase 1 ----
            kcT1 = pool.tile([128, 512], BF16, tag="kcT1", name="kcT1")
            nc.scalar.copy(kcT1[:], ps_kc1[:])
            for c in range(4):
                nc.tensor.transpose(trall2[:, c, :],
                                    kcT1[:, c * 128:(c + 1) * 128], id_v)
            # separate PSUM tile (trall2): a second read of the same PSUM
            # tile needs two sync waits, which compute instrs can't encode
            kc_hi = pool.tile([128, 4, 128], BF16, tag="kchi", name="kchi")
            nc.vector.tensor_copy(
                kc_hi[:].rearrange("p c s -> p (c s)"),
                trall2[:].rearrange("p c s -> p (c s)"))

            # ---- S2 partial 1 ----
            s2p1_last = None
            for c in range(4):
                s2p1_last = nc.tensor.matmul(ps_db[:], kc_hi[:, c, :],
                                             dbt_hi[:, c, :],
                                             start=False, stop=(c == 3))

            # duplicate C into partitions 64-127 (partition-shifted copy)
            nc.vector.tensor_copy(g3b_v[64:128, :], g3b_v[0:64, :])

            # ---- PW in [s, b] layout (no transposes) ----
            db_sb = pool.tile([128, 2 * B], BF16, tag="db", name="db")
            nc.vector.tensor_copy(db_sb[:], ps_db[:])
            # partition-swapped copy of the B half (tensor_tensor requires
            # same start partition on all APs; tensor_copy does not)
            dbsw = pool.tile([128, B], BF16, tag="dbsw", name="dbsw")
            nc.vector.tensor_copy(dbsw[0:64, :], db_sb[64:128, B:2 * B])
            nc.vector.tensor_copy(dbsw[64:128, :], db_sb[0:64, B:2 * B])
            ptA = pool.tile([128, B], BF16, tag="ptA", name="ptA")
            ptC2 = pool.tile([128, B], BF16, tag="ptC2", name="ptC2")
            nc.vector.tensor_mul(ptA[:], db_sb[:, 0:B], db_sb[:, B:2 * B])
            nc.vector.tensor_mul(ptC2[:], db_sb[:, 0:B], dbsw[:])

            # hold the PE clock through the PW (DVE) window; pinned after
            # S2p1 so these cannot float back into the DMA stream
            jh = None
            for w in range(3):
                jh = nc.tensor.matmul(ps_junk[:, 0:256], wz[:, :128],
                                      wz[:, 128:384], start=True, stop=True)
                if w == 0:
                    add_dep_helper(jh.ins, s2p1_last.ins, sync=False,
                                   reason="order: junk after S2p1")

            # ---- S4 per bank: out[b, j] = ptA^T G3a + ptC2^T G3b2 ----
            out_lo = pool.tile([128, 512], BF16, tag="outlo", name="outlo")
            out_hi = pool.tile([128, 512], BF16, tag="outhi", name="outhi")
            stores = []
            # A-mms first (ptA ready before ptC2; one stationary switch),
            # C-mms close each bank's accumulation group
            nc.tensor.matmul(ps_out_lo[:], ptA[:], g3a_v[:, 0:512],
                             start=True, stop=False)
            nc.tensor.matmul(ps_out_hi[:], ptA[:], g3a_v[:, 512:1024],
                             start=True, stop=False)
            nc.tensor.matmul(ps_out_lo[:], ptC2[:], g3b_v[:, 0:512],
                             start=False, stop=True)
            cp_lo = nc.scalar.copy(out_lo[:], ps_out_lo[:])
            stores.append(nc.sync.dma_start(out[:, :256],
                                            out_lo.bitcast(F32)[:, :]))
            last_mm = nc.tensor.matmul(ps_out_hi[:], ptC2[:],
                                       g3b_v[:, 512:1024],
                                       start=False, stop=True)
            cp_hi = nc.vector.tensor_copy(out_hi[:], ps_out_hi[:])
            stores.append(nc.scalar.dma_start(out[:, 256:],
                                              out_hi.bitcast(F32)[:, :]))

            # ---- tail: absorb every outstanding tick into SP's clock ----
            prev = None
            for dep in [*in_dmas, memset_h, *stores, last_mm, cp_lo, cp_hi]:
                dr = nc.sync.drain(fusable=False)
                add_dep_helper(dr.ins, dep.ins, sync=True,
                               reason="tail: absorb tick into SP clock")
                if prev is not None:
                    add_dep_helper(dr.ins, prev.ins, sync=False,
                                   reason="tail: keep drain chain ordered")
                prev = dr

    return nc


def _bf16_pack(a):
    """float32 (P, W) -> bf16 packed two-per-word as float32 (P, W//2)."""
    bf = np.ascontiguousarray(np.asarray(a, np.float32).astype(ml_dtypes.bfloat16))
    return bf.view(np.uint8).reshape(bf.shape[0], -1).view(np.float32)


def _partition_pack(a):
    """(n*128, W) -> (128, n*W): row p = concat of chunk rows p."""
    r, w = a.shape
    n = r // 128
    return np.ascontiguousarray(
        a.reshape(n, 128, w).transpose(1, 0, 2).reshape(128, n * w))


def _constants():
    """Per-core CC [N, S], G3a [128, N], G3b2 [128, N] float32."""
    j = np.arange(N, dtype=np.float64)
    alt = np.cos(np.pi * j)                     # (-1)^j
    ccs, g3as, g3bs = [], [], []
    for c in range(N_CORES):
        f = np.arange(c * FPC, (c + 1) * FPC, dtype=np.float64)
        ang = 2.0 * np.pi * np.outer(j, f) / N             # (j, t)
        cc_re = np.cos(ang)
        cc_im = -np.sin(ang)
        angT = ang.T                                        # (t, j)
        w = 4.0 / N
        A = w * np.cos(angT)                                # m0 rows
        Bm = -w * np.cos(angT)                              # m1 rows
        C = -w * np.sin(angT)                               # mC rows
        if c == 0:
            cc_im[:, 0] = alt                               # f=512 cos column
            A[0, :] = 2.0 / N                               # m0 = D0*B0
            Bm[0, :] = (2.0 / N) * alt                      # m1 = D512*B512
            C[0, :] = 0.0
        cc_full = np.concatenate([cc_re, cc_im], axis=1)    # (N, 128)
        ccs.append(np.ascontiguousarray(cc_full, np.float32))
        g3as.append(np.ascontiguousarray(
            np.concatenate([A, Bm], axis=0), np.float32))       # (128, N)
        g3bs.append(np.ascontiguousarray(
            np.concatenate([C, C], axis=0), np.float32))        # (128, N)
    return ccs, g3as, g3bs


def kernel(des, body, kernel):
    global LAST_RESULT
    K = np.asarray(kernel, dtype=np.float32)
    des = np.asarray(des, dtype=np.float32)
    body = np.asarray(body, dtype=np.float32)

    # K^T as bf16 blocks: block (c, h) = K^T[c*128:(c+1)*128, h*512:(h+1)*512]
    ktb = K.T.astype(ml_dtypes.bfloat16)                # (1024 j, 1024 k)
    def ktpk(c, h):
        blk = np.ascontiguousarray(
            ktb[c * 128:(c + 1) * 128, h * 512:(h + 1) * 512], np.float32)
        return _bf16_pack(blk)                          # (128, 256) words

    id_pk = _bf16_pack(np.eye(128, dtype=np.float32))   # (128, 64) words
    dbt_np = np.concatenate([des.T, body.T], axis=1)    # (1024, 256)
    dbt_pk = _partition_pack(_bf16_pack(dbt_np))        # (128, 1024) words

    ccs, g3as, g3bs = _constants()
    in_maps = []
    for c in range(N_CORES):
        cc_pk = _partition_pack(_bf16_pack(ccs[c]))     # (128, 512) words
        m = {
            "sp1": np.ascontiguousarray(np.concatenate(
                [id_pk, ktpk(0, 0), ktpk(1, 0), ktpk(2, 0), ktpk(3, 0)],
                axis=1)),
            "sp2": np.ascontiguousarray(np.concatenate(
                [ktpk(0, 1), ktpk(1, 1), ktpk(2, 1), ktpk(3, 1)], axis=1)),
            "ac1": np.ascontiguousarray(np.concatenate(
                [ktpk(4, 0), ktpk(5, 0), ktpk(6, 0), ktpk(7, 0)], axis=1)),
            "ac2": np.ascontiguousarray(np.concatenate(
                [ktpk(4, 1), ktpk(5, 1), ktpk(6, 1), ktpk(7, 1),
                 dbt_pk], axis=1)),
            "cc": cc_pk,
            "g3a": np.ascontiguousarray(_bf16_pack(g3as[c])),
            "g3b": np.ascontiguousarray(_bf16_pack(g3bs[c][0:64])),
        }
        in_maps.append(m)

    if "nc" not in _nc_cache:
        _nc_cache["nc"] = _build_nc()
    nc = _nc_cache["nc"]

    res = run_bass_kernel_spmd(nc, in_maps, list(range(N_CORES)))
    LAST_RESULT = res
    out = np.zeros((B, N), dtype=np.float32)
    for r in res.results:
        w = np.ascontiguousarray(np.asarray(r["out"], np.float32))
        bf = w.view(np.uint8).reshape(B, -1).view(ml_dtypes.bfloat16)
        out += bf.astype(np.float32)
    return out


# revision 22
# speedup vs baseline: 1.1422x; 1.0665x over previous
r"""Circulant layer kernel for Trainium2 (8 NeuronCores) — v7.

Math (same as v2): reference computes mv1 + mv2 = 2 * circconv(d, b)
with d = des @ K, b = body @ K.  Real-input half-spectrum DFT: cores
0..7 own freqs f = 64c..64c+63; Nyquist f=512 rides core 0's slot-0
imaginary column with the generalized 3-product inverse (G3).

v7 vs v3: the input stream is ordered by when each tensor is needed.
dbt (needed at S2 partial 1, ~24us) rides the kt h1 HWDGE DMAs; g3a/g3b
(needed only at S4, ~26us) move to the slow SWDGE queue behind cc.
Keeping non-DMA engines QUIET during the stream matters: junk matmuls /
generation ALU measurably throttle DMA ingress (245 -> 140GB/s).

v3 structural changes vs v2 (40.9us -> 38.1 measured):
  * K^T streams on BOTH hardware DMA queues (SP + ACT), split by
    j-chunk pairs and k-halves; cc/dbt ride the gpsimd SWDGE queue.
    (v2 put all of kt on one queue at ~190GB/s — the single-queue
    stream, not PE, set the critical path.)
  * k-half phasing: S1 (KC^T = CC^T K^T) accumulates k-half 0 in PSUM
    bank 0 and k-half 1 in bank 1, so T1/S2 for half 0 run while
    half 1 is still streaming in.
  * The pointwise spectral products are computed directly in [s, b]
    layout from S2's output (DVE ops with partition-base-shifted
    operands — verified on HW), eliminating T2, T3 and their staging
    copies entirely:
      ptA[p, b]        = db[p, b] * db[p, B+b]          (p = 0..127)
      ptC2[p, b]       = db[p, b] * db[(p+64)%128, B+b] (two half ops)
    ptA/ptC2 feed S4 as stationaries with G3a / duplicated-C moving.
  * S4 + cast + store issue per 512-col PSUM bank as soon as ready.

Fixed costs measured by probe: ~8.3us preamble, ~2us DMA issue->land,
~2us store issue->tick, ~8.3us after last store tick.
"""

import numpy as np
import ml_dtypes

import concourse.bass as bass
import concourse.mybir as mybir
import concourse.tile as tile
from concourse.bass_utils import run_bass_kernel_spmd
from concourse.tile_rust import add_dep_helper

B = 128        # batch
D_IN = 1024    # input feature dim (contraction k)
N = 1024       # output feature dim (conv length j)
N_CORES = 8
FPC = 64       # complex frequency slots per core
S = 2 * FPC    # 128 freq columns per core: [0:64]=re(cos), [64:128]=im(-sin)

F32 = mybir.dt.float32
BF16 = mybir.dt.bfloat16

LAST_RESULT = None
_nc_cache = {}


def _build_nc():
    nc = bass.Bass(target_bir_lowering=True)

    # --- DRAM params (bf16 packed two-per-f32-word) ---
    # SP queue: [id | kt h0 c0c1] [kt h0 c2c3] [kt h1 c0c1] [kt h1 c2c3] [g3a]
    # ACT queue: [kt h0 c4c5] [kt h0 c6c7] [kt h1 c4c5] [kt h1 c6c7] [g3b2]
    # GP queue: [cc] [dbt]
    # each kt (c,h) block: [128, 512] bf16 = 256 f32 words; pairs = 512 words
    sp1 = nc.declare_dram_parameter("sp1", [128, 64 + 1024], F32, False)
    sp2 = nc.declare_dram_parameter("sp2", [128, 1024], F32, False)
    ac1 = nc.declare_dram_parameter("ac1", [128, 1024], F32, False)
    ac2 = nc.declare_dram_parameter("ac2", [128, 2048], F32, False)
    cc = nc.declare_dram_parameter("cc", [128, 512], F32, False)
    g3a = nc.declare_dram_parameter("g3a", [128, 512], F32, False)
    g3b = nc.declare_dram_parameter("g3b", [64, 512], F32, False)
    out = nc.declare_dram_parameter("out", [B, N // 2], F32, isOutput=True)

    with tile.TileContext(nc) as tc:
        with (
            tc.tile_pool(name="main", bufs=1) as pool,
            tc.tile_pool(name="psum", bufs=1, space="PSUM") as pp,
        ):
            # ---- input DMAs, phase-ordered per queue ----
            sp1_sb = pool.tile([128, 64 + 1024], F32, tag="sp1", name="sp1")
            sp2_sb = pool.tile([128, 1024], F32, tag="sp2", name="sp2")
            ac1_sb = pool.tile([128, 1024], F32, tag="ac1", name="ac1")
            ac2_sb = pool.tile([128, 2048], F32, tag="ac2", name="ac2")
            cc_sb = pool.tile([128, 512], F32, tag="cc", name="cc")
            g3a_sb = pool.tile([128, 512], F32, tag="g3a", name="g3a")
            g3b_sb = pool.tile([128, 512], F32, tag="g3b", name="g3b")

            in_dmas = []
            in_dmas.append(nc.sync.dma_start(sp1_sb[:], sp1[:, :]))
            in_dmas.append(nc.sync.dma_start(sp2_sb[:], sp2[:, :]))
            in_dmas.append(nc.sync.dma_start(g3a_sb[:], g3a[:, :]))
            in_dmas.append(nc.scalar.dma_start(ac1_sb[:], ac1[:, :]))
            in_dmas.append(nc.scalar.dma_start(ac2_sb[:], ac2[:, :]))
            in_dmas.append(nc.gpsimd.dma_start(cc_sb[:], cc[:, :]))
            in_dmas.append(nc.gpsimd.dma_start(g3b_sb[0:64, :], g3b[:, :]))

            # bf16 views
            id_v = sp1_sb.bitcast(BF16)[:, 0:128]
            # kt[c][h] -> [128, 512] bf16 view
            sp1v = sp1_sb.bitcast(BF16)
            sp2v = sp2_sb.bitcast(BF16)
            ac1v = ac1_sb.bitcast(BF16)
            ac2v = ac2_sb.bitcast(BF16)
            ktv = {}
            for c in range(4):
                ktv[(c, 0)] = sp1v[:, 128 + c * 512:128 + (c + 1) * 512]
                ktv[(c, 1)] = sp2v[:, c * 512:(c + 1) * 512]
                ktv[(4 + c, 0)] = ac1v[:, c * 512:(c + 1) * 512]
                ktv[(4 + c, 1)] = ac2v[:, c * 512:(c + 1) * 512]
            g3a_v = g3a_sb.bitcast(BF16)          # [128, 1024]
            # g3b = [C; C]: only rows 0-63 are DMA'd; duplicate on DVE
            g3b_v = g3b_sb.bitcast(BF16)          # [128, 1024]
            cc_v = cc_sb.bitcast(BF16).rearrange(
                "p (c s) -> p c s", c=8)          # [128, 8, 128]
            # dbt rides the tail of ac2 (all 8 k-chunks)
            dbt_lo = ac2v[:, 2048:3072].rearrange(
                "p (c w) -> p c w", c=4)          # [128, 4, 256]
            dbt_hi = ac2v[:, 3072:4096].rearrange(
                "p (c w) -> p c w", c=4)

            # ---- PSUM layout ----
            ps_kc0 = pp.tile([128, 512], F32, tag="pskc0", name="pskc0")
            ps_kc1 = pp.tile([128, 512], F32, tag="pskc1", name="pskc1")
            ps_db = pp.tile([128, 2 * B], F32, tag="psdb", name="psdb")
            trall = pp.tile([128, 4, 128], BF16, tag="trall", name="trall")
            trall2 = pp.tile([128, 4, 128], BF16, tag="trall2", name="trall2")
            ps_out_lo = pp.tile([128, 512], F32, tag="psoutl", name="psoutl")
            ps_out_hi = pp.tile([128, 512], F32, tag="psouth", name="psouth")
            ps_junk = pp.tile([128, 512], F32, tag="psjunk", name="psjunk")

            # ---- PE warmup: junk matmuls into ps_out (S4 overwrites) ----
            wz = pool.tile([128, 640], BF16, tag="wz", name="wz")
            memset_h = nc.gpsimd.memset(wz[:], 0.0)
            # single PE wake-up mm (more junk here runs during the DMA
            # stream and measurably slows ingress)
            nc.tensor.matmul(ps_junk[:], wz[:, :128], wz[:, 128:640],
                             start=True, stop=True)

            # ---- S1 phase 0: ps_kc0[s, k0:512] = sum_j cc[j,s]^T kt[j, h0] ----
            # mm order follows expected landing: SP pair (0,1), ACT (4,5),
            # SP (2,3), ACT (6,7)
            h0_order = [0, 1, 4, 5, 2, 3, 6, 7]
            for i, c in enumerate(h0_order):
                nc.tensor.matmul(ps_kc0[:], cc_v[:, c, :], ktv[(c, 0)],
                                 start=(i == 0), stop=(i == 7))

            # ---- T1 phase 0: transpose KC^T[:, 0:512] -> kc chunks 0..3 ----
            kcT0 = pool.tile([128, 512], BF16, tag="kcT0", name="kcT0")
            nc.scalar.copy(kcT0[:], ps_kc0[:])
            tr_h0_last = None
            for c in range(4):
                tr_h0_last = nc.tensor.transpose(
                    trall[:, c, :], kcT0[:, c * 128:(c + 1) * 128], id_v)
            kc_lo = pool.tile([128, 4, 128], BF16, tag="kclo", name="kclo")
            nc.vector.tensor_copy(
                kc_lo[:].rearrange("p c s -> p (c s)"),
                trall[:].rearrange("p c s -> p (c s)"))

            # ---- S1 phase 1 (first half): overlap with S2p0 setup ----
            h1_order = [0, 1, 4, 5, 2, 3, 6, 7]
            for i, c in enumerate(h1_order[:2]):
                nc.tensor.matmul(ps_kc1[:], cc_v[:, c, :], ktv[(c, 1)],
                                 start=(i == 0), stop=False)

            # ---- S2 partial 0: ps_db += kc[k0 chunks] @ dbt ----
            for c in range(4):
                nc.tensor.matmul(ps_db[:], kc_lo[:, c, :], dbt_lo[:, c, :],
                                 start=(c == 0), stop=False)

            # ---- S1 phase 1 (rest) ----
            for i, c in enumerate(h1_order[2:]):
                nc.tensor.matmul(ps_kc1[:], cc_v[:, c, :], ktv[(c, 1)],
                                 start=False, stop=(i == 5))

            # ---- T1 phase 1 ----
            kcT1 = pool.tile([128, 512], BF16, tag="kcT1", name="kcT1")
            nc.scalar.copy(kcT1[:], ps_kc1[:])
            for c in range(4):
                nc.tensor.transpose(trall2[:, c, :],
                                    kcT1[:, c * 128:(c + 1) * 128], id_v)
            # separate PSUM tile (trall2): a second read of the same PSUM
            # tile needs two sync waits, which compute instrs can't encode
            kc_hi = pool.tile([128, 4, 128], BF16, tag="kchi", name="kchi")
            nc.vector.tensor_copy(
                kc_hi[:].rearrange("p c s -> p (c s)"),
                trall2[:].rearrange("p c s -> p (c s)"))

            # ---- S2 partial 1 ----
            s2p1_last = None
            for c in range(4):
                s2p1_last = nc.tensor.matmul(ps_db[:], kc_hi[:, c, :],
                                             dbt_hi[:, c, :],
                                             start=False, stop=(c == 3))

            # duplicate C into partitions 64-127 (partition-shifted copy)
            nc.vector.tensor_copy(g3b_v[64:128, :], g3b_v[0:64, :])

            # ---- PW in [s, b] layout (no transposes) ----
            db_sb = pool.tile([128, 2 * B], BF16, tag="db", name="db")
            nc.vector.tensor_copy(db_sb[:], ps_db[:])
            # partition-swapped copy of the B half (tensor_tensor requires
            # same start partition on all APs; tensor_copy does not)
            dbsw = pool.tile([128, B], BF16, tag="dbsw", name="dbsw")
            nc.vector.tensor_copy(dbsw[0:64, :], db_sb[64:128, B:2 * B])
            nc.vector.tensor_copy(dbsw[64:128, :], db_sb[0:64, B:2 * B])
            ptA = pool.tile([128, B], BF16, tag="ptA", name="ptA")
            ptC2 = pool.tile([128, B], BF16, tag="ptC2", name="ptC2")
            nc.vector.tensor_mul(ptA[:], db_sb[:, 0:B], db_sb[:, B:2 * B])
            nc.vector.tensor_mul(ptC2[:], db_sb[:, 0:B], dbsw[:])

            # hold the PE clock through the PW (DVE) window; pinned after
            # S2p1 so these cannot float back into the DMA stream
            jh = None
            for w in range(3):
                jh = nc.tensor.matmul(ps_junk[:, 0:256], wz[:, :128],
                                      wz[:, 128:384], start=True, stop=True)
                if w == 0:
                    add_dep_helper(jh.ins, s2p1_last.ins, sync=False,
                                   reason="order: junk after S2p1")

            # ---- S4 per bank: out[b, j] = ptA^T G3a + ptC2^T G3b2 ----
            out_lo = pool.tile([128, 512], BF16, tag="outlo", name="outlo")
            out_hi = pool.tile([128, 512], BF16, tag="outhi", name="outhi")
            stores = []
            # A-mms first (ptA ready before ptC2; one stationary switch),
            # C-mms close each bank's accumulation group
            nc.tensor.matmul(ps_out_lo[:], ptA[:], g3a_v[:, 0:512],
                             start=True, stop=False)
            nc.tensor.matmul(ps_out_hi[:], ptA[:], g3a_v[:, 512:1024],
                             start=True, stop=False)
            nc.tensor.matmul(ps_out_lo[:], ptC2[:], g3b_v[:, 0:512],
                             start=False, stop=True)
            cp_lo = nc.scalar.copy(out_lo[:], ps_out_lo[:])
            stores.append(nc.sync.dma_start(out[:, :256],
                                            out_lo.bitcast(F32)[:, :]))
            last_mm = nc.tensor.matmul(ps_out_hi[:], ptC2[:],
                                       g3b_v[:, 512:1024],
                                       start=False, stop=True)
            cp_hi = nc.vector.tensor_copy(out_hi[:], ps_out_hi[:])
            stores.append(nc.scalar.dma_start(out[:, 256:],
                                              out_hi.bitcast(F32)[:, :]))

            # ---- tail: absorb every outstanding tick into SP's clock ----
            prev = None
            for dep in [*in_dmas, memset_h, *stores, last_mm, cp_lo, cp_hi]:
                dr = nc.sync.drain(fusable=False)
                add_dep_helper(dr.ins, dep.ins, sync=True,
                               reason="tail: absorb tick into SP clock")
                if prev is not None:
                    add_dep_helper(dr.ins, prev.ins, sync=False,
                                   reason="tail: keep drain chain ordered")
                prev = dr

    return nc


def _bf16_pack(a):
    """float32 (P, W) -> bf16 packed two-per-word as float32 (P, W//2)."""
    bf = np.ascontiguousarray(np.asarray(a, np.float32).astype(ml_dtypes.bfloat16))
    return bf.view(np.uint8).reshape(bf.shape[0], -1).view(np.float32)


def _partition_pack(a):
    """(n*128, W) -> (128, n*W): row p = concat of chunk rows p."""
    r, w = a.shape
    n = r // 128
    return np.ascontiguousarray(
        a.reshape(n, 128, w).transpose(1, 0, 2).reshape(128, n * w))


def _constants():
    """Per-core CC [N, S], G3a [128, N], G3b2 [128, N] float32."""
    j = np.arange(N, dtype=np.float64)
    alt = np.cos(np.pi * j)                     # (-1)^j
    ccs, g3as, g3bs = [], [], []
    for c in range(N_CORES):
        f = np.arange(c * FPC, (c + 1) * FPC, dtype=np.float64)
        ang = 2.0 * np.pi * np.outer(j, f) / N             # (j, t)
        cc_re = np.cos(ang)
        cc_im = -np.sin(ang)
        angT = ang.T                                        # (t, j)
        w = 4.0 / N
        A = w * np.cos(angT)                                # m0 rows
        Bm = -w * np.cos(angT)                              # m1 rows
        C = -w * np.sin(angT)                               # mC rows
        if c == 0:
            cc_im[:, 0] = alt                               # f=512 cos column
            A[0, :] = 2.0 / N                               # m0 = D0*B0
            Bm[0, :] = (2.0 / N) * alt                      # m1 = D512*B512
            C[0, :] = 0.0
        cc_full = np.concatenate([cc_re, cc_im], axis=1)    # (N, 128)
        ccs.append(np.ascontiguousarray(cc_full, np.float32))
        g3as.append(np.ascontiguousarray(
            np.concatenate([A, Bm], axis=0), np.float32))       # (128, N)
        g3bs.append(np.ascontiguousarray(
            np.concatenate([C, C], axis=0), np.float32))        # (128, N)
    return ccs, g3as, g3bs


def kernel(des, body, kernel):
    global LAST_RESULT
    K = np.asarray(kernel, dtype=np.float32)
    des = np.asarray(des, dtype=np.float32)
    body = np.asarray(body, dtype=np.float32)

    # K^T as bf16 blocks: block (c, h) = K^T[c*128:(c+1)*128, h*512:(h+1)*512]
    ktb = K.T.astype(ml_dtypes.bfloat16)                # (1024 j, 1024 k)
    def ktpk(c, h):
        blk = np.ascontiguousarray(
            ktb[c * 128:(c + 1) * 128, h * 512:(h + 1) * 512], np.float32)
        return _bf16_pack(blk)                          # (128, 256) words

    id_pk = _bf16_pack(np.eye(128, dtype=np.float32))   # (128, 64) words
    dbt_np = np.concatenate([des.T, body.T], axis=1)    # (1024, 256)
    dbt_pk = _partition_pack(_bf16_pack(dbt_np))        # (128, 1024) words

    ccs, g3as, g3bs = _constants()
    in_maps = []
    for c in range(N_CORES):
        cc_pk = _partition_pack(_bf16_pack(ccs[c]))     # (128, 512) words
        m = {
            "sp1": np.ascontiguousarray(np.concatenate(
                [id_pk, ktpk(0, 0), ktpk(1, 0), ktpk(2, 0), ktpk(3, 0)],
                axis=1)),
            "sp2": np.ascontiguousarray(np.concatenate(
                [ktpk(0, 1), ktpk(1, 1), ktpk(2, 1), ktpk(3, 1)], axis=1)),
            "ac1": np.ascontiguousarray(np.concatenate(
                [ktpk(4, 0), ktpk(5, 0), ktpk(6, 0), ktpk(7, 0)], axis=1)),
            "ac2": np.ascontiguousarray(np.concatenate(
                [ktpk(4, 1), ktpk(5, 1), ktpk(6, 1), ktpk(7, 1),
                 dbt_pk], axis=1)),
            "cc": cc_pk,
            "g3a": np.ascontiguousarray(_bf16_pack(g3as[c])),
            "g3b": np.ascontiguousarray(_bf16_pack(g3bs[c][0:64])),
        }
        in_maps.append(m)

    if "nc" not in _nc_cache:
        _nc_cache["nc"] = _build_nc()
    nc = _nc_cache["nc"]

    res = run_bass_kernel_spmd(nc, in_maps, list(range(N_CORES)))
    LAST_RESULT = res
    out = np.zeros((B, N), dtype=np.float32)
    for r in res.results:
        w = np.ascontiguousarray(np.asarray(r["out"], np.float32))
        bf = w.view(np.uint8).reshape(B, -1).view(ml_dtypes.bfloat16)
        out += bf.astype(np.float32)
    return out
